# revision 43
# baseline (speedup 1.0000x reference)
"""Dense GAT layer (nn_DenseGATLayer) Trainium2 Bass kernel.

Problem (per batch b of B=8):
    Wh   = X[b] @ W                                   [N=1024, H*F=256]
    s[n,h] = <Wh[n,h,:], a_src[h]>,  d[n,h] = <Wh[n,h,:], a_dst[h]>
    e[i,j,h] = lrelu(s[i,h] + d[j,h], 0.2);  masked by A[b,i,j]
    alpha = softmax_j(e);  out[i,h,:] = elu(sum_j alpha[i,j,h] Wh[j,h,:])

Sharding: data-parallel, one batch per NeuronCore (B=8 == n_cores=8).

Per-core pipeline (transposed-score layout, fully fused):
  - A[b] (int32 0/1) -> SWDGE cast-DMA to int16 in HBM (8 column chunks),
    then HWDGE DMA-transposed (2-byte xbar) into SBUF as AT16[j,i] tiles.
    The mask never touches a compute engine.
  - s,d in 1/2048 fixed point via one small matmul with host-prepacked
    Wa = (W @ blockdiag(a_src,a_dst)) * 2048 -> eT[8,1024]. s rows are
    replicated over partitions with K=1 ones-matmuls on TensorE; d
    columns come from tiny PE transposes.
  - One custom DVE op per (head, source-tile) computes the fully masked
    leaky-relu score t = (AT16-1)*BIG + s16 + d ; e = max(t, 0.2t) in a
    single pass. ScalarE exponentiates 4 tiles at a time
    (exp(e/2048), bf16 out).
  - TensorE accumulates P^T-slices @ [Wh_h | 1] over source tiles in
    PSUM; col 64 of each accumulator is the softmax denominator Z.
  - Batched normalize + exact ELU per head: rz = 1/Z (one strided DVE
    reciprocal), u = acc*rz (DVE, broadcast AP), ev = exp(u) (ScalarE),
    out = u >= 0 ? u : ev - 1 (custom DVE select).
"""

import sys

if "/opt/trn_rl_repo" not in sys.path:
    sys.path.insert(0, "/opt/trn_rl_repo")

from contextlib import ExitStack

import numpy as np

import concourse.bass as bass
import concourse.tile as tile
from concourse import bacc, mybir
from concourse import bass_utils
from concourse._compat import with_exitstack
from concourse.masks import make_identity

# ------------------------------------------------------------------ params
B, N, DIN, H, F = 8, 1024, 256, 4, 64
HF = H * F
NT = N // 128            # 8 node tiles
KT = DIN // 128          # 2 contraction tiles
SCALE = 2048.0           # fixed-point scale for s/d scores
BIG = 1.0e6              # mask offset in fixed-point units (-> -488 real)
LRELU_ALPHA = 0.2

dt = mybir.dt
AF = mybir.ActivationFunctionType

# ------------------------------------------------------------- custom DVE ops
from concourse.dve_ops import (
    DveOp,
    OPS,
    _SUB_OPCODE_FOR_NAME,
    CUSTOM_DVE_SPECS,
    _CUSTOM_DVE_ROW_BASE,
)
from concourse.dve_spec import (
    Spec,
    Src0,
    Src1,
    C0,
    C1,
    C2,
    Zero,
    One,
    lower,
    maxx,
    select,
    _has_src1,
)
from concourse.dve_uop import DveOpSpec


def _register_op(name, spec):
    for o in OPS:
        if o.name == name:
            return o
    opcode = _CUSTOM_DVE_ROW_BASE + len(OPS)
    shas = {}
    for ver in ("v3", "v4"):
        s = DveOpSpec(
            name=name, opcode=opcode, uops=lower(spec, ver=ver), rd1_en=_has_src1(spec)
        )
        shas[ver] = s.sha(ver)
    op = DveOp(name, spec, subdim=False, uops_sha=shas)
    OPS.append(op)
    _SUB_OPCODE_FOR_NAME[name] = opcode
    CUSTOM_DVE_SPECS[name] = spec
    return op


def _score_ref(in0, in1, s0, s1, imm2):
    t = (np.asarray(in0, np.float32) - 1) * imm2 + np.asarray(in1, np.float32) + s0
    return np.maximum(t, t * s1)


# masked leaky-relu score: t = (in0-1)*imm2 + (in1+s0); out = max(t, t*s1)
_t = (Src0 - One) * C2 + (Src1 + C0)
GAT_SCORE = _register_op(
    "GAT_SCORE_ANT",
    Spec(body=maxx(_t, _t * C1), reference=_score_ref),
)

# variant with hoisted constant: t = in0*imm2 + (in1 + (s0 - imm2))
_t2 = Src0 * C2 + (Src1 + (C0 - C2))
GAT_SCORE2 = _register_op(
    "GAT_SCORE2_ANT",
    Spec(body=maxx(_t2, _t2 * C1), reference=_score_ref),
)

# elu select: out = in0 >= 0 ? in0 : in1 - 1
GAT_SEL2 = _register_op(
    "GAT_SEL2_ANT",
    Spec(
        body=select(Src0 >= Zero, Src0, Src1 - One),
        reference=lambda in0, in1, s0, s1, imm2: np.where(in0 >= 0, in0, in1 - 1),
    ),
)


def _bcast_last(ap, n):
    """Append a step-0 free dim of size n to an AP (broadcast along it)."""
    return bass.AP(ap.tensor, ap.offset, [list(d) for d in ap.ap] + [[0, n]])


# ------------------------------------------------------------------ kernel body
@with_exitstack
def _gat_body(ctx: ExitStack, tc: "tile.TileContext", Xd, Ad, Wd, Wad, IDENTd, SELd, OUTd):
    nc = tc.nc
    f32, bf16, i16 = dt.float32, dt.bfloat16, dt.int16

    sb = ctx.enter_context(tc.tile_pool(name="sb", bufs=1))
    dram = ctx.enter_context(tc.tile_pool(name="dram", bufs=1, space="DRAM"))

    # ---------- loads: X in one strided DMA on sync, W + Wa on scalar ----------
    Xsb = sb.tile([128, NT * DIN], f32)  # node tile it at cols [it*DIN, ...)
    x_insts = [
        nc.sync.dma_start(
            Xsb[:].rearrange("p (it c) -> p it c", it=NT),
            Xd[:].rearrange("(it p) c -> p it c", p=128),
        )
    ]

    # identity + one-hot s-broadcast selectors (tiny constant inputs)
    ident = sb.tile([128, 128], f32)
    nc.sync.dma_start(ident[:], IDENTd[:])
    sel = sb.tile([8, H * 128], f32)
    nc.sync.dma_start(sel[:], SELd[:])

    # ---------- A cast to int16 in HBM (4 contiguous row chunks); gated on the
    # X loads so the small X transfer is not starved by the 6MB cast traffic --
    a16d = dram.tile([N, N], i16)
    for c in range(4):
        ci = nc.gpsimd.dma_start(
            a16d[c * 256 : (c + 1) * 256, :], Ad[c * 256 : (c + 1) * 256, :]
        )
        for xi in x_insts:
            tile.add_dep_helper(ci.ins, xi.ins, reason="prioritize X over A cast")
    Wsb = sb.tile([128, KT * HF], f32)
    Wasb = sb.tile([128, KT * 2 * H], f32)
    for kt in range(KT):
        nc.scalar.dma_start(
            Wsb[:, kt * HF : (kt + 1) * HF], Wd[kt * 128 : (kt + 1) * 128, :]
        )
        nc.scalar.dma_start(
            Wasb[:, kt * 2 * H : (kt + 1) * 2 * H], Wad[kt * 128 : (kt + 1) * 128, :]
        )

    # warm the exp activation table before it is on the critical path
    scrap = sb.tile([1, 1], f32)
    nc.gpsimd.memset(scrap[:], 0.0)
    nc.scalar.activation(scrap[:], scrap[:], AF.Exp)

    # ---------- A transposes: one [1024, 128] xbar transpose per source tile.
    # Finer grain costs ~0.5us extra fixed overhead total but delivers tile 0
    # ~2.5us earlier, and the stream stays ahead of score consumption.
    AT16 = sb.tile([128, NT * N], i16)  # tile jt at cols [jt*N, (jt+1)*N)
    for jt in range(NT):
        nc.sync.dma_start_transpose(
            AT16[:, jt * N : (jt + 1) * N], a16d[:, jt * 128 : (jt + 1) * 128]
        )

    XTsb = sb.tile([128, KT * N], f32)  # XT[p=chan, kt*N + node]
    Whb = sb.tile([128, NT * H * (F + 1)], bf16)  # [p=node, jt, h, f|1]
    w4 = Whb[:].rearrange("p (jt h f) -> p jt h f", jt=NT, h=H)
    eTf = sb.tile([8, N], f32)
    dcols = sb.tile([128, NT * 2 * H], f32)  # per jt: 2H cols (2h=src, 2h+1=dst)
    s16 = sb.tile([128, H * N], i16)

    with (
        tc.tile_pool(name="psT", bufs=2, space="PSUM") as psT,
        tc.tile_pool(name="psW", bufs=2, space="PSUM") as psW,
        tc.tile_pool(name="psE", bufs=1, space="PSUM") as psE,
    ):
        # X^T via PE transposes; copies on DVE (idle early)
        for it in range(NT):
            for kt in range(KT):
                pt = psT.tile([128, 128], f32, tag="pt")
                nc.tensor.transpose(
                    pt[:],
                    Xsb[:, it * DIN + kt * 128 : it * DIN + (kt + 1) * 128],
                    ident[:],
                )
                nc.vector.tensor_copy(
                    XTsb[:, kt * N + it * 128 : kt * N + (it + 1) * 128], pt[:]
                )

        # eT = Wa^T @ X^T   ([8, N], fp32, scaled by 2048)
        pe = psE.tile([8, N], f32)
        for nh in range(2):
            for kt in range(KT):
                nc.tensor.matmul(
                    pe[:, nh * 512 : (nh + 1) * 512],
                    Wasb[:, kt * 2 * H : (kt + 1) * 2 * H],
                    XTsb[:, kt * N + nh * 512 : kt * N + (nh + 1) * 512],
                    start=(kt == 0),
                    stop=(kt == KT - 1),
                )
        nc.scalar.copy(eTf[:], pe[:])

        # d columns: tiny PE transposes of eTf 128-col slabs
        for jt in range(NT):
            pd = psT.tile([128, 8], f32, tag="pt")
            nc.tensor.transpose(pd[:], eTf[:, jt * 128 : (jt + 1) * 128], ident[0:8, 0:8])
            nc.vector.tensor_copy(dcols[:, jt * 2 * H : (jt + 1) * 2 * H], pd[:])

        # s broadcast: one-hot selector matmul replicates eTf row 2h to all
        # partitions (K=8), then fixed-point int16 copy on ScalarE
        for h in range(H):
            pb = psW.tile([128, N], f32, tag="pw")
            for nh in range(2):
                nc.tensor.matmul(
                    pb[:, nh * 512 : (nh + 1) * 512],
                    sel[:, h * 128 : (h + 1) * 128],
                    eTf[:, nh * 512 : (nh + 1) * 512],
                    start=True,
                    stop=True,
                )
            nc.scalar.copy(s16[:, h * N : (h + 1) * N], pb[:])



        # Wh (node-major, bf16, strided per-head layout with ones column)
        nc.gpsimd.memset(w4[:, :, :, F], 1.0)
        for it in range(NT):
            pw = psW.tile([128, HF], f32, tag="pw")
            for kt in range(KT):
                nc.tensor.matmul(
                    pw[:],
                    XTsb[:, kt * N + it * 128 : kt * N + (it + 1) * 128],
                    Wsb[:, kt * HF : (kt + 1) * HF],
                    start=(kt == 0),
                    stop=(kt == KT - 1),
                )
            nc.scalar.copy(w4[:, it, :, 0:F], pw[:].rearrange("p (h f) -> p h f", h=H))

    # ---------- head loop: scores -> exp -> AV matmul -> normalize+elu --------
    OutSB = sb.tile([128, NT * HF], f32)
    Out4 = OutSB[:].rearrange("p (it h f) -> p it h f", it=NT, h=H)
    psA = ctx.enter_context(tc.tile_pool(name="psA", bufs=3, space="PSUM"))
    pts = ctx.enter_context(tc.tile_pool(name="pts", bufs=4))
    eps = ctx.enter_context(tc.tile_pool(name="eps", bufs=6))
    small = ctx.enter_context(tc.tile_pool(name="small", bufs=3))

    for h in range(H):
        PTh = pts.tile([128, NT * N], bf16, tag="pt")
        # scores; exp batched at FD=2048, except the last head where FD=1024
        # shortens the tail chain into the final AV accumulation
        expw = 1 if h == H - 1 else 2
        for jp in range(NT // expw):
            ep = eps.tile([128, 2 * N], f32, tag="ep")
            for k in range(expw):
                jt = expw * jp + k
                nc.vector._custom_dve(
                    GAT_SCORE2,
                    out=ep[:, k * N : (k + 1) * N],
                    in0=AT16[:, jt * N : (jt + 1) * N],
                    in1=s16[:, h * N : (h + 1) * N],
                    s0=dcols[:, jt * 2 * H + 2 * h + 1 : jt * 2 * H + 2 * h + 2],
                    s1=LRELU_ALPHA,
                    imm2=BIG,
                )
            nc.scalar.activation(
                PTh[:, jp * expw * N : (jp + 1) * expw * N],
                ep[:, 0 : expw * N],
                AF.Exp,
                scale=1.0 / SCALE,
            )
        # attention-weighted values; Z rides along in column F of each block
        acc8 = psA.tile([128, NT, 128], f32, tag="acc")
        for it in range(NT):
            for jt in range(NT):
                nc.tensor.matmul(
                    acc8[:, it, 0 : F + 1],
                    PTh[:, jt * N + it * 128 : jt * N + (it + 1) * 128],
                    w4[:, jt, h, :],
                    start=(jt == 0),
                    stop=(jt == NT - 1),
                )
        # batched normalize + exact elu, split in it-halves so the first half
        # starts as soon as its AV chains retire
        ngroups = 2
        gsz = NT // ngroups
        for g in range(ngroups):
            its = slice(g * gsz, (g + 1) * gsz)
            rz = small.tile([128, gsz], f32, tag="rz")
            nc.vector.reciprocal(rz[:], acc8[:, its, F])
            u8 = small.tile([128, gsz, F], f32, tag="u8")
            nc.vector.tensor_tensor(
                u8[:], acc8[:, its, 0:F], _bcast_last(rz[:], F), op=mybir.AluOpType.mult
            )
            ev8 = small.tile([128, gsz, F], f32, tag="ev8")
            nc.scalar.activation(ev8[:], u8[:], AF.Exp)
            nc.vector._custom_dve(GAT_SEL2, out=Out4[:, its, h, :], in0=u8[:], in1=ev8[:])

    for it in range(NT):
        eng = nc.sync if it % 2 == 0 else nc.scalar
        eng.dma_start(
            OUTd[it * 128 : (it + 1) * 128, :], OutSB[:, it * HF : (it + 1) * HF]
        )


# ------------------------------------------------------------------ build/run
_NC_CACHE = {}


def _build_nc():
    if "nc" in _NC_CACHE:
        return _NC_CACHE["nc"]
    nc = bacc.Bacc(
        "TRN2",
        target_bir_lowering=False,
        debug=False,
        enable_asserts=False,
        num_devices=B,
    )
    Xd = nc.dram_tensor("X", [N, DIN], dt.float32, kind="ExternalInput").ap()
    Ad = nc.dram_tensor("A", [N, N], dt.int32, kind="ExternalInput").ap()
    Wd = nc.dram_tensor("W", [DIN, HF], dt.float32, kind="ExternalInput").ap()
    Wad = nc.dram_tensor("Wa", [DIN, 2 * H], dt.float32, kind="ExternalInput").ap()
    IDENTd = nc.dram_tensor("IDENT", [128, 128], dt.float32, kind="ExternalInput").ap()
    SELd = nc.dram_tensor("SEL", [8, H * 128], dt.float32, kind="ExternalInput").ap()
    OUTd = nc.dram_tensor("OUT", [N, HF], dt.float32, kind="ExternalOutput").ap()
    with tile.TileContext(nc) as tc:
        _gat_body(tc, Xd, Ad, Wd, Wad, IDENTd, SELd, OUTd)
    nc.compile()
    _NC_CACHE["nc"] = nc
    return nc


def _host_prep(W, a_src, a_dst):
    Wh_w = np.asarray(W, np.float32).reshape(DIN, H, F)
    Wa = np.empty((DIN, 2 * H), np.float32)
    Wa[:, 0::2] = np.einsum("khf,hf->kh", Wh_w, np.asarray(a_src, np.float32))
    Wa[:, 1::2] = np.einsum("khf,hf->kh", Wh_w, np.asarray(a_dst, np.float32))
    return Wa * SCALE


def _run(X, A, W, a_src, a_dst, **spmd_kwargs):
    X = np.ascontiguousarray(np.asarray(X, np.float32))
    A = np.ascontiguousarray(np.asarray(A, np.int32))
    W = np.ascontiguousarray(np.asarray(W, np.float32))
    Wa = _host_prep(W, a_src, a_dst)
    nc = _build_nc()
    ident = np.eye(128, dtype=np.float32)
    sel = np.zeros((8, H * 128), np.float32)
    for h in range(H):
        sel[2 * h, h * 128 : (h + 1) * 128] = 1.0
    in_maps = [
        {"X": X[b], "A": A[b], "W": W, "Wa": Wa, "IDENT": ident, "SEL": sel}
        for b in range(B)
    ]
    res = bass_utils.run_bass_kernel_spmd(
        nc, in_maps, core_ids=list(range(B)), **spmd_kwargs
    )
    out = np.stack([np.asarray(res.results[b]["OUT"]) for b in range(B)])
    return out.astype(np.float32), res


def kernel(X, A, W, a_src, a_dst):
    out, _ = _run(X, A, W, a_src, a_dst)
    return out


if __name__ == "__main__":
    rng = np.random.default_rng(0)
    out = kernel(
        X=rng.standard_normal((B, N, DIN)).astype(np.float32),
        A=rng.integers(0, 2, size=(B, N, N)).astype(np.int32),
        W=(rng.standard_normal((DIN, HF)) * 0.06).astype(np.float32),
        a_src=(rng.standard_normal((H, F)) * 0.17).astype(np.float32),
        a_dst=(rng.standard_normal((H, F)) * 0.17).astype(np.float32),
    )
    print(out.shape, out.dtype)


# revision 44
# speedup vs baseline: 1.0440x; 1.0440x over previous
"""Dense GAT layer (nn_DenseGATLayer) Trainium2 Bass kernel.

Problem (per batch b of B=8):
    Wh   = X[b] @ W                                   [N=1024, H*F=256]
    s[n,h] = <Wh[n,h,:], a_src[h]>,  d[n,h] = <Wh[n,h,:], a_dst[h]>
    e[i,j,h] = lrelu(s[i,h] + d[j,h], 0.2);  masked by A[b,i,j]
    alpha = softmax_j(e);  out[i,h,:] = elu(sum_j alpha[i,j,h] Wh[j,h,:])

Sharding: data-parallel, one batch per NeuronCore (B=8 == n_cores=8).

Per-core pipeline (transposed-score layout, fully fused):
  - A[b] (int32 0/1) -> SWDGE cast-DMA to int16 in HBM (8 column chunks),
    then HWDGE DMA-transposed (2-byte xbar) into SBUF as AT16[j,i] tiles.
    The mask never touches a compute engine.
  - s,d in 1/2048 fixed point via one small matmul with host-prepacked
    Wa = (W @ blockdiag(a_src,a_dst)) * 2048 -> eT[8,1024]. s rows are
    replicated over partitions with K=1 ones-matmuls on TensorE; d
    columns come from tiny PE transposes.
  - One custom DVE op per (head, source-tile) computes the fully masked
    leaky-relu score t = (AT16-1)*BIG + s16 + d ; e = max(t, 0.2t) in a
    single pass. ScalarE exponentiates 4 tiles at a time
    (exp(e/2048), bf16 out).
  - TensorE accumulates P^T-slices @ [Wh_h | 1] over source tiles in
    PSUM; col 64 of each accumulator is the softmax denominator Z.
  - Batched normalize + exact ELU per head: rz = 1/Z (one strided DVE
    reciprocal), u = acc*rz (DVE, broadcast AP), ev = exp(u) (ScalarE),
    out = u >= 0 ? u : ev - 1 (custom DVE select).
"""

import sys

if "/opt/trn_rl_repo" not in sys.path:
    sys.path.insert(0, "/opt/trn_rl_repo")

from contextlib import ExitStack

import numpy as np

import concourse.bass as bass
import concourse.tile as tile
from concourse import bacc, mybir
from concourse import bass_utils
from concourse._compat import with_exitstack
from concourse.masks import make_identity

# ------------------------------------------------------------------ params
B, N, DIN, H, F = 8, 1024, 256, 4, 64
HF = H * F
NT = N // 128            # 8 node tiles
KT = DIN // 128          # 2 contraction tiles
SCALE = 2048.0           # fixed-point scale for s/d scores
BIG = 1.0e6              # mask offset in fixed-point units (-> -488 real)
LRELU_ALPHA = 0.2

dt = mybir.dt
AF = mybir.ActivationFunctionType

# ------------------------------------------------------------- custom DVE ops
from concourse.dve_ops import (
    DveOp,
    OPS,
    _SUB_OPCODE_FOR_NAME,
    CUSTOM_DVE_SPECS,
    _CUSTOM_DVE_ROW_BASE,
)
from concourse.dve_spec import (
    Spec,
    Src0,
    Src1,
    C0,
    C1,
    C2,
    Zero,
    One,
    lower,
    maxx,
    select,
    _has_src1,
)
from concourse.dve_uop import DveOpSpec


def _register_op(name, spec):
    for o in OPS:
        if o.name == name:
            return o
    opcode = _CUSTOM_DVE_ROW_BASE + len(OPS)
    shas = {}
    for ver in ("v3", "v4"):
        s = DveOpSpec(
            name=name, opcode=opcode, uops=lower(spec, ver=ver), rd1_en=_has_src1(spec)
        )
        shas[ver] = s.sha(ver)
    op = DveOp(name, spec, subdim=False, uops_sha=shas)
    OPS.append(op)
    _SUB_OPCODE_FOR_NAME[name] = opcode
    CUSTOM_DVE_SPECS[name] = spec
    return op


def _score_ref(in0, in1, s0, s1, imm2):
    t = (np.asarray(in0, np.float32) - 1) * imm2 + np.asarray(in1, np.float32) + s0
    return np.maximum(t, t * s1)


# masked leaky-relu score: t = (in0-1)*imm2 + (in1+s0); out = max(t, t*s1)
_t = (Src0 - One) * C2 + (Src1 + C0)
GAT_SCORE = _register_op(
    "GAT_SCORE_ANT",
    Spec(body=maxx(_t, _t * C1), reference=_score_ref),
)

# variant with hoisted constant: t = in0*imm2 + (in1 + (s0 - imm2))
_t2 = Src0 * C2 + (Src1 + (C0 - C2))
GAT_SCORE2 = _register_op(
    "GAT_SCORE2_ANT",
    Spec(body=maxx(_t2, _t2 * C1), reference=_score_ref),
)

# elu select: out = in0 >= 0 ? in0 : in1 - 1
GAT_SEL2 = _register_op(
    "GAT_SEL2_ANT",
    Spec(
        body=select(Src0 >= Zero, Src0, Src1 - One),
        reference=lambda in0, in1, s0, s1, imm2: np.where(in0 >= 0, in0, in1 - 1),
    ),
)


def _bcast_last(ap, n):
    """Append a step-0 free dim of size n to an AP (broadcast along it)."""
    return bass.AP(ap.tensor, ap.offset, [list(d) for d in ap.ap] + [[0, n]])


# ------------------------------------------------------------------ kernel body
@with_exitstack
def _gat_body(ctx: ExitStack, tc: "tile.TileContext", Xd, Ad, Wd, Wad, IDENTd, SELd, OUTd):
    nc = tc.nc
    f32, bf16, i16 = dt.float32, dt.bfloat16, dt.int16

    sb = ctx.enter_context(tc.tile_pool(name="sb", bufs=1))
    dram = ctx.enter_context(tc.tile_pool(name="dram", bufs=1, space="DRAM"))

    # ---------- loads: X in one strided DMA on sync, W + Wa on scalar ----------
    Xsb = sb.tile([128, NT * DIN], f32)  # node tile it at cols [it*DIN, ...)
    x_insts = [
        nc.sync.dma_start(
            Xsb[:].rearrange("p (it c) -> p it c", it=NT),
            Xd[:].rearrange("(it p) c -> p it c", p=128),
        )
    ]

    # identity + one-hot s-broadcast selectors (tiny constant inputs)
    ident = sb.tile([128, 128], f32)
    nc.sync.dma_start(ident[:], IDENTd[:])
    sel = sb.tile([8, H * 128], f32)
    nc.sync.dma_start(sel[:], SELd[:])

    # ---------- A cast to int16 in HBM (4 contiguous row chunks); gated on the
    # X loads so the small X transfer is not starved by the 6MB cast traffic --
    a16d = dram.tile([N, N], i16)
    for c in range(4):
        ci = nc.gpsimd.dma_start(
            a16d[c * 256 : (c + 1) * 256, :], Ad[c * 256 : (c + 1) * 256, :]
        )
        for xi in x_insts:
            tile.add_dep_helper(ci.ins, xi.ins, reason="prioritize X over A cast")
    Wsb = sb.tile([128, KT * HF], f32)
    Wasb = sb.tile([128, KT * 2 * H], f32)
    for kt in range(KT):
        nc.scalar.dma_start(
            Wsb[:, kt * HF : (kt + 1) * HF], Wd[kt * 128 : (kt + 1) * 128, :]
        )
        nc.scalar.dma_start(
            Wasb[:, kt * 2 * H : (kt + 1) * 2 * H], Wad[kt * 128 : (kt + 1) * 128, :]
        )

    # warm the exp activation table before it is on the critical path
    scrap = sb.tile([1, 1], f32)
    nc.gpsimd.memset(scrap[:], 0.0)
    nc.scalar.activation(scrap[:], scrap[:], AF.Exp)

    # ---------- A transposes: one [1024, 128] xbar transpose per source tile.
    # Finer grain costs ~0.5us extra fixed overhead total but delivers tile 0
    # ~2.5us earlier, and the stream stays ahead of score consumption.
    AT16 = sb.tile([128, NT * N], i16)  # tile jt at cols [jt*N, (jt+1)*N)
    for jt in range(NT):
        nc.sync.dma_start_transpose(
            AT16[:, jt * N : (jt + 1) * N], a16d[:, jt * 128 : (jt + 1) * 128]
        )

    XTsb = sb.tile([128, KT * N], f32)  # XT[p=chan, kt*N + node]
    Whb = sb.tile([128, NT * H * (F + 1)], bf16)  # [p=node, jt, h, f|1]
    w4 = Whb[:].rearrange("p (jt h f) -> p jt h f", jt=NT, h=H)
    eTf = sb.tile([8, N], f32)
    dcols = sb.tile([128, NT * 2 * H], f32)  # per jt: 2H cols (2h=src, 2h+1=dst)
    s16 = sb.tile([128, H * N], i16)

    with (
        tc.tile_pool(name="psT", bufs=2, space="PSUM") as psT,
        tc.tile_pool(name="psW", bufs=2, space="PSUM") as psW,
        tc.tile_pool(name="psE", bufs=1, space="PSUM") as psE,
    ):
        # X^T via PE transposes; copies on DVE (idle early)
        for it in range(NT):
            for kt in range(KT):
                pt = psT.tile([128, 128], f32, tag="pt")
                nc.tensor.transpose(
                    pt[:],
                    Xsb[:, it * DIN + kt * 128 : it * DIN + (kt + 1) * 128],
                    ident[:],
                )
                nc.vector.tensor_copy(
                    XTsb[:, kt * N + it * 128 : kt * N + (it + 1) * 128], pt[:]
                )

        # eT = Wa^T @ X^T   ([8, N], fp32, scaled by 2048)
        pe = psE.tile([8, N], f32)
        for nh in range(2):
            for kt in range(KT):
                nc.tensor.matmul(
                    pe[:, nh * 512 : (nh + 1) * 512],
                    Wasb[:, kt * 2 * H : (kt + 1) * 2 * H],
                    XTsb[:, kt * N + nh * 512 : kt * N + (nh + 1) * 512],
                    start=(kt == 0),
                    stop=(kt == KT - 1),
                )
        nc.scalar.copy(eTf[:], pe[:])

        # d columns: tiny PE transposes of eTf 128-col slabs
        for jt in range(NT):
            pd = psT.tile([128, 8], f32, tag="pt")
            nc.tensor.transpose(pd[:], eTf[:, jt * 128 : (jt + 1) * 128], ident[0:8, 0:8])
            nc.vector.tensor_copy(dcols[:, jt * 2 * H : (jt + 1) * 2 * H], pd[:])

        # s broadcast: one-hot selector matmul replicates eTf row 2h to all
        # partitions (K=8), then fixed-point int16 copy on ScalarE
        for h in range(H):
            pb = psW.tile([128, N], f32, tag="pw")
            for nh in range(2):
                nc.tensor.matmul(
                    pb[:, nh * 512 : (nh + 1) * 512],
                    sel[:, h * 128 : (h + 1) * 128],
                    eTf[:, nh * 512 : (nh + 1) * 512],
                    start=True,
                    stop=True,
                )
            nc.scalar.copy(s16[:, h * N : (h + 1) * N], pb[:])



        # Wh (node-major, bf16, strided per-head layout with ones column)
        nc.gpsimd.memset(w4[:, :, :, F], 1.0)
        for it in range(NT):
            pw = psW.tile([128, HF], f32, tag="pw")
            for kt in range(KT):
                nc.tensor.matmul(
                    pw[:],
                    XTsb[:, kt * N + it * 128 : kt * N + (it + 1) * 128],
                    Wsb[:, kt * HF : (kt + 1) * HF],
                    start=(kt == 0),
                    stop=(kt == KT - 1),
                )
            nc.scalar.copy(w4[:, it, :, 0:F], pw[:].rearrange("p (h f) -> p h f", h=H))

    # ---------- head loop: scores -> exp -> AV matmul -> normalize+elu --------
    OutSB = sb.tile([128, NT * HF], f32)
    Out4 = OutSB[:].rearrange("p (it h f) -> p it h f", it=NT, h=H)
    psA = ctx.enter_context(tc.tile_pool(name="psA", bufs=2, space="PSUM"))
    pts = ctx.enter_context(tc.tile_pool(name="pts", bufs=3))
    eps = ctx.enter_context(tc.tile_pool(name="eps", bufs=4))
    small = ctx.enter_context(tc.tile_pool(name="small", bufs=3))

    for h in range(H):
        PTh = pts.tile([128, NT * N], bf16, tag="pt")
        # scores; exp batched at FD=2048, except the last head where FD=1024
        # shortens the tail chain into the final AV accumulation
        expw = 1 if h == H - 1 else 2
        for jp in range(NT // expw):
            ep = eps.tile([128, 2 * N], f32, tag="ep")
            for k in range(expw):
                jt = expw * jp + k
                nc.vector._custom_dve(
                    GAT_SCORE2,
                    out=ep[:, k * N : (k + 1) * N],
                    in0=AT16[:, jt * N : (jt + 1) * N],
                    in1=s16[:, h * N : (h + 1) * N],
                    s0=dcols[:, jt * 2 * H + 2 * h + 1 : jt * 2 * H + 2 * h + 2],
                    s1=LRELU_ALPHA,
                    imm2=BIG,
                )
            nc.scalar.activation(
                PTh[:, jp * expw * N : (jp + 1) * expw * N],
                ep[:, 0 : expw * N],
                AF.Exp,
                scale=1.0 / SCALE,
            )
        # attention-weighted values; Z rides along in column F of each block
        acc8 = psA.tile([128, NT, 128], f32, tag="acc")
        for it in range(NT):
            for jt in range(NT):
                nc.tensor.matmul(
                    acc8[:, it, 0 : F + 1],
                    PTh[:, jt * N + it * 128 : jt * N + (it + 1) * 128],
                    w4[:, jt, h, :],
                    start=(jt == 0),
                    stop=(jt == NT - 1),
                )
        # batched normalize + exact elu, split in it-halves so the first half
        # starts as soon as its AV chains retire
        ngroups = 2
        gsz = NT // ngroups
        for g in range(ngroups):
            its = slice(g * gsz, (g + 1) * gsz)
            rz = small.tile([128, gsz], f32, tag="rz")
            nc.vector.reciprocal(rz[:], acc8[:, its, F])
            u8 = small.tile([128, gsz, F], f32, tag="u8")
            nc.vector.tensor_tensor(
                u8[:], acc8[:, its, 0:F], _bcast_last(rz[:], F), op=mybir.AluOpType.mult
            )
            ev8 = small.tile([128, gsz, F], f32, tag="ev8")
            nc.scalar.activation(ev8[:], u8[:], AF.Exp)
            nc.vector._custom_dve(GAT_SEL2, out=Out4[:, its, h, :], in0=u8[:], in1=ev8[:])

    for it in range(NT):
        eng = nc.sync if it % 2 == 0 else nc.scalar
        eng.dma_start(
            OUTd[it * 128 : (it + 1) * 128, :], OutSB[:, it * HF : (it + 1) * HF]
        )


# ------------------------------------------------------------------ build/run
_NC_CACHE = {}


def _build_nc():
    if "nc" in _NC_CACHE:
        return _NC_CACHE["nc"]
    nc = bacc.Bacc(
        "TRN2",
        target_bir_lowering=False,
        debug=False,
        enable_asserts=False,
        num_devices=B,
    )
    Xd = nc.dram_tensor("X", [N, DIN], dt.float32, kind="ExternalInput").ap()
    Ad = nc.dram_tensor("A", [N, N], dt.int32, kind="ExternalInput").ap()
    Wd = nc.dram_tensor("W", [DIN, HF], dt.float32, kind="ExternalInput").ap()
    Wad = nc.dram_tensor("Wa", [DIN, 2 * H], dt.float32, kind="ExternalInput").ap()
    IDENTd = nc.dram_tensor("IDENT", [128, 128], dt.float32, kind="ExternalInput").ap()
    SELd = nc.dram_tensor("SEL", [8, H * 128], dt.float32, kind="ExternalInput").ap()
    OUTd = nc.dram_tensor("OUT", [N, HF], dt.float32, kind="ExternalOutput").ap()
    with tile.TileContext(nc) as tc:
        _gat_body(tc, Xd, Ad, Wd, Wad, IDENTd, SELd, OUTd)
    nc.compile()
    _NC_CACHE["nc"] = nc
    return nc


def _host_prep(W, a_src, a_dst):
    Wh_w = np.asarray(W, np.float32).reshape(DIN, H, F)
    Wa = np.empty((DIN, 2 * H), np.float32)
    Wa[:, 0::2] = np.einsum("khf,hf->kh", Wh_w, np.asarray(a_src, np.float32))
    Wa[:, 1::2] = np.einsum("khf,hf->kh", Wh_w, np.asarray(a_dst, np.float32))
    return Wa * SCALE


def _run(X, A, W, a_src, a_dst, **spmd_kwargs):
    X = np.ascontiguousarray(np.asarray(X, np.float32))
    A = np.ascontiguousarray(np.asarray(A, np.int32))
    W = np.ascontiguousarray(np.asarray(W, np.float32))
    Wa = _host_prep(W, a_src, a_dst)
    nc = _build_nc()
    ident = np.eye(128, dtype=np.float32)
    sel = np.zeros((8, H * 128), np.float32)
    for h in range(H):
        sel[2 * h, h * 128 : (h + 1) * 128] = 1.0
    in_maps = [
        {"X": X[b], "A": A[b], "W": W, "Wa": Wa, "IDENT": ident, "SEL": sel}
        for b in range(B)
    ]
    res = bass_utils.run_bass_kernel_spmd(
        nc, in_maps, core_ids=list(range(B)), **spmd_kwargs
    )
    out = np.stack([np.asarray(res.results[b]["OUT"]) for b in range(B)])
    return out.astype(np.float32), res


def kernel(X, A, W, a_src, a_dst):
    out, _ = _run(X, A, W, a_src, a_dst)
    return out


if __name__ == "__main__":
    rng = np.random.default_rng(0)
    out = kernel(
        X=rng.standard_normal((B, N, DIN)).astype(np.float32),
        A=rng.integers(0, 2, size=(B, N, N)).astype(np.int32),
        W=(rng.standard_normal((DIN, HF)) * 0.06).astype(np.float32),
        a_src=(rng.standard_normal((H, F)) * 0.17).astype(np.float32),
        a_dst=(rng.standard_normal((H, F)) * 0.17).astype(np.float32),
    )
    print(out.shape, out.dtype)
